# revision 6
# baseline (speedup 1.0000x reference)
"""Trainium2 Bass kernel for nn_ClusterFeatureBranch (GAT x2 + GCN + MLP branches).

Sharding: nodes (and their in-edges, edge-cut by dst) are partitioned across
8 NeuronCores; small weight matrices are replicated; per-layer transformed
node features are AllGathered so every core can gather messages for its own
edges from local HBM.

Per GAT/GCN layer, per core:
  - prep: xw = h @ W (PE), attention logits a_src/a_dst per node (DVE),
    pack to bf16, DMA to DRAM shard, AllGather -> full [N, 520] bf16.
  - aggregate, per 128-dst-node window: indirect-DMA gather of per-edge
    message rows (by src id), gather of per-edge alpha_dst (by dst id),
    softmax over dst segments via one-hot matmuls (scatter-add on PE into
    PSUM), normalize + bias + relu on DVE.
Softmax max-subtraction is skipped (m=0): logits here are O(0.1), so
exp never overflows and the result is mathematically identical.
"""

import os
import sys

for _p in ("/opt/trn_rl_repo",):
    if os.path.isdir(_p) and _p not in sys.path:
        sys.path.append(_p)

import numpy as np
import ml_dtypes

from concourse import bacc, bass, mybir, tile
from concourse.bass_utils import run_bass_kernel_spmd
from concourse.masks import make_identity

P = 128
H, F = 4, 128
HF = H * F            # 512
ROW = HF + 8          # gather row: 512 feat bf16 + 4 alpha_src + 4 pad
G = 8
PCA, POS = 256, 128
OUTD = HF + PCA + POS  # 896
BN_EPS = 1e-5
NCORES = 8

f32 = mybir.dt.float32
bf16 = mybir.dt.bfloat16
i32 = mybir.dt.int32
bf16np = ml_dtypes.bfloat16


# ---------------------------------------------------------------- host prep

def _padT(a, cols):
    out = np.zeros((a.shape[1], cols), np.float32)
    out[:, :a.shape[0]] = a.T
    return out


def host_prep(x, edge_index, batch, scales, n_cores=NCORES):
    """Pure index/layout preprocessing (the sharding step). Returns per-core
    tensors + shared scalars. All float math stays on device."""
    N = x.shape[0]
    npc = N // n_cores
    W = (npc + P - 1) // P

    src = np.asarray(edge_index[0], np.int64)
    dst = np.asarray(edge_index[1], np.int64)
    loops = np.arange(N, dtype=np.int64)
    src = np.concatenate([src, loops])
    dst = np.concatenate([dst, loops])

    # gcn norm (pure graph structure): deg includes self loop
    deg = np.bincount(dst, minlength=N).astype(np.float64)
    dis = np.where(deg > 0, 1.0 / np.sqrt(deg), 0.0)
    enorm = (dis[src] * dis[dst]).astype(np.float32)

    order = np.argsort(dst, kind="stable")
    src_s, dst_s, enorm_s = src[order], dst[order], enorm[order]

    # split per core / per window, find uniform B
    counts = np.zeros((n_cores, W), np.int64)
    core_of = dst_s // npc
    win_of = (dst_s - core_of * npc) // P
    for c in range(n_cores):
        m = core_of == c
        cw = win_of[m]
        for w in range(W):
            counts[c, w] = int((cw == w).sum())
    B = int(max(1, int(np.ceil(counts.max() / P))))
    EPW = B * P

    per_core = []
    gcounts = np.bincount(np.asarray(batch, np.int64), minlength=G).astype(np.float32)
    inv_cnt = (1.0 / np.maximum(gcounts, 1.0)).astype(np.float32).reshape(G, 1)

    for c in range(n_cores):
        m = core_of == c
        cs, cd, cn = src_s[m], dst_s[m], enorm_s[m]
        cw = win_of[m]
        idx_src = np.zeros((P, W * B), np.int32)
        idx_dst = np.zeros((P, W * B), np.int32)
        S_gat = np.zeros((W * B, P, P), np.float32)
        S_gcn = np.zeros((W * B, P, P), np.float32)
        for w in range(W):
            wm = cw == w
            es, ed, en = cs[wm], cd[wm], cn[wm]
            k = len(es)
            assert k <= EPW
            e = np.arange(k)
            b, p = e // P, e % P
            blk = w * B + b
            idx_src[p, blk] = es
            idx_dst[p, blk] = (ed - c * npc).astype(np.int32)
            nloc = (ed - c * npc - w * P).astype(np.int64)
            S_gat[blk, p, nloc] = 1.0
            S_gcn[blk, p, nloc] = en
        # batch one-hot selector [P, W*G] and xT/sT shards
        sel = np.zeros((P, W * G), np.float32)
        nb = np.asarray(batch, np.int64)[c * npc:(c + 1) * npc]
        for w in range(W):
            rows = min(P, npc - w * P)
            sel[np.arange(rows), w * G + nb[w * P:w * P + rows]] = 1.0
        per_core.append(dict(
            idx_src=idx_src,
            idx_dst=idx_dst,
            S_gat=S_gat.astype(bf16np),
            S_gcn=S_gcn.astype(bf16np),
            sel=sel,
            xT=_padT(np.asarray(x, np.float32)[c * npc:(c + 1) * npc], W * P),
            sT=_padT(np.asarray(scales, np.float32)[c * npc:(c + 1) * npc], W * P),
        ))
    return per_core, inv_cnt, npc, W, B


# ---------------------------------------------------------------- program

def build_program(N, npc, W, B, weights_meta):
    """Build the single-program-multiple-data Bass module."""
    nc = bacc.Bacc()
    NPAD = W * P

    # ---- I/O
    inp = {}
    def din(name, shape, dt):
        inp[name] = nc.declare_dram_parameter(name, list(shape), dt, isOutput=False)
        return inp[name]

    din("idx_src", (P, W * B), i32)
    din("idx_dst", (P, W * B), i32)
    din("S_gat", (W * B, P, P), bf16)
    din("S_gcn", (W * B, P, P), bf16)
    din("sel", (P, W * G), f32)
    din("xT", (9, W * P), f32)
    din("sT", (1, W * P), f32)
    din("inv_cnt", (G, 1), f32)
    for name, shape in weights_meta:
        din(name, shape, f32)

    hcomb = nc.declare_dram_parameter("hcomb", [npc, OUTD], f32, isOutput=True)
    frag = nc.declare_dram_parameter("frag", [G, OUTD], f32, isOutput=True)

    # ---- internal DRAM
    xw1_sh = nc.dram_tensor("xw1_sh", [npc, ROW], bf16)
    xw2_sh = nc.dram_tensor("xw2_sh", [npc, ROW], bf16)
    xw3_sh = nc.dram_tensor("xw3_sh", [npc, HF], bf16)
    xw1_ext = nc.dram_tensor("xw1_ext", [N, ROW], bf16, addr_space="Shared")
    xw2_ext = nc.dram_tensor("xw2_ext", [N, ROW], bf16, addr_space="Shared")
    xw3_ext = nc.dram_tensor("xw3_ext", [N, HF], bf16, addr_space="Shared")
    ad1 = nc.dram_tensor("ad1", [NPAD, 8], f32)
    ad2 = nc.dram_tensor("ad2", [NPAD, 8], f32)
    frag_in = nc.dram_tensor("frag_in", [G, OUTD], f32)
    frag_out = nc.dram_tensor("frag_out", [G, OUTD], f32, addr_space="Shared")

    RG = [list(range(NCORES))]

    with tile.TileContext(nc) as tc:
        with (
            tc.tile_pool(name="const", bufs=1) as cpool,
            tc.tile_pool(name="resid", bufs=1) as rpool,
            tc.tile_pool(name="work", bufs=2) as wpool,
            tc.tile_pool(name="sc", bufs=3) as scpool,
            tc.tile_pool(name="ps", bufs=4, space="PSUM") as pspool,
            tc.tile_pool(name="psfrag", bufs=1, space="PSUM") as fragpool,
        ):
            # ---------------- constants
            ident = cpool.tile([P, P], f32)
            make_identity(nc, ident[:])
            ones = cpool.tile([1, P], f32)
            nc.vector.memset(ones[:], 1.0)

            def rep_row(name, d, cast=None, scale=None):
                """replicate a [1, d] dram row across 128 partitions"""
                row = rpool.tile([1, d], f32, tag="reprow")
                nc.sync.dma_start(out=row[:], in_=inp[name][:])
                out = cpool.tile([P, d], f32, tag=f"rep_{name}")
                for j in range(0, d, 512):
                    e = min(512, d - j)
                    pr = pspool.tile([P, 512], f32, tag="ps")
                    nc.tensor.matmul(pr[:, :e], lhsT=ones[:], rhs=row[:, j:j + e],
                                     start=True, stop=True)
                    nc.vector.tensor_copy(out=out[:, j:j + e], in_=pr[:, :e])
                if scale is not None:
                    nc.vector.tensor_scalar_mul(out[:], out[:], scale)
                return out

            bns = float(1.0 / np.sqrt(1.0 + BN_EPS))
            b1_r = rep_row("b1", HF)
            b2_r = rep_row("b2", HF)
            b3_r = rep_row("b3", HF)
            ac1_r = rep_row("a_comb1", 2 * HF)
            ac2_r = rep_row("a_comb2", 2 * HF)
            g1_r = rep_row("g1", 128, scale=bns)
            be1_r = rep_row("be1", 128)
            pb1_r = rep_row("pb1", 128)
            g2_r = rep_row("g2", 256, scale=bns)
            be2_r = rep_row("be2", 256)
            pb2_r = rep_row("pb2", 256)
            pb3_r = rep_row("pb3", 256)
            sb1_r = rep_row("sb1", 32)
            sb2_r = rep_row("sb2", 128)
            # BN folded consts: c = pb*s + be
            c1_r = cpool.tile([P, 128], f32)
            nc.vector.tensor_mul(c1_r[:], pb1_r[:], g1_r[:])
            nc.vector.tensor_add(c1_r[:], c1_r[:], be1_r[:])
            c2_r = cpool.tile([P, 256], f32)
            nc.vector.tensor_mul(c2_r[:], pb2_r[:], g2_r[:])
            nc.vector.tensor_add(c2_r[:], c2_r[:], be2_r[:])

            # ---------------- resident loads
            hslab = rpool.tile([P, W * HF], f32, tag="hslab")
            idxs_sb = rpool.tile([P, W * B], i32, tag="idxs")
            nc.gpsimd.dma_start(out=idxs_sb[:], in_=inp["idx_src"][:])
            idxd_sb = rpool.tile([P, W * B], i32, tag="idxd")
            nc.gpsimd.dma_start(out=idxd_sb[:], in_=inp["idx_dst"][:])
            sel_sb = rpool.tile([P, W * G], f32, tag="sel")
            nc.sync.dma_start(out=sel_sb[:], in_=inp["sel"][:])
            inv_sb = rpool.tile([G, 1], f32, tag="invc")
            nc.sync.dma_start(out=inv_sb[:], in_=inp["inv_cnt"][:])

            def load_w_bf16(name, kchunks, ncols):
                out = rpool.tile([P, kchunks * ncols], bf16, tag=f"w_{name}")
                for hh in range(2):
                    tmp = wpool.tile([P, kchunks * ncols // 2], f32, tag="asd_tmp")
                    nc.sync.dma_start(
                        out=tmp[:],
                        in_=inp[name][:].rearrange("(c p) n -> p c n", p=P)
                            [:, hh * kchunks // 2:(hh + 1) * kchunks // 2, :])
                    nc.vector.tensor_copy(
                        out=out[:, hh * kchunks * ncols // 2:
                                (hh + 1) * kchunks * ncols // 2], in_=tmp[:])
                return out

            W2_sb = load_w_bf16("W2", 4, HF)
            W3_sb = load_w_bf16("W3", 4, HF)
            W1_sb = rpool.tile([9, HF], f32, tag="W1")
            nc.sync.dma_start(out=W1_sb[:], in_=inp["W1"][:])
            pW2_sb = rpool.tile([P, PCA], f32, tag="pW2")
            nc.sync.dma_start(out=pW2_sb[:], in_=inp["pW2"][:])
            pW3_sb = rpool.tile([P, 2 * PCA], f32, tag="pW3")
            nc.sync.dma_start(
                out=pW3_sb[:].rearrange("p (c n) -> p c n", c=2),
                in_=inp["pW3"][:].rearrange("(c p) n -> p c n", p=P))
            pW1_sb = rpool.tile([9, 128], f32, tag="pW1")
            nc.sync.dma_start(out=pW1_sb[:], in_=inp["pW1"][:])
            sW1_sb = rpool.tile([1, 32], f32, tag="sW1")
            nc.sync.dma_start(out=sW1_sb[:], in_=inp["sW1"][:])
            sW2_sb = rpool.tile([32, 128], f32, tag="sW2")
            nc.sync.dma_start(out=sW2_sb[:], in_=inp["sW2"][:])
            xT_sb = rpool.tile([9, W * P], f32, tag="xT")
            nc.sync.dma_start(out=xT_sb[:], in_=inp["xT"][:])
            sT_sb = rpool.tile([1, W * P], f32, tag="sT")
            nc.sync.dma_start(out=sT_sb[:], in_=inp["sT"][:])

            def rows_of(t):
                return min(P, npc - t * P)

            # -------- attention logit helper: asd [P, 8] from xw psum
            def alpha_sd(xw_ps, ac_rep):
                tmp = wpool.tile([P, 2 * HF], f32, tag="asd_tmp")
                nc.vector.tensor_mul(tmp[:, 0:HF], xw_ps[:], ac_rep[:, 0:HF])
                nc.vector.tensor_mul(tmp[:, HF:], xw_ps[:], ac_rep[:, HF:])
                asd = wpool.tile([P, 8], f32, tag="asd")
                nc.vector.reduce_sum(
                    out=asd[:],
                    in_=tmp[:].rearrange("p (c f) -> p c f", f=F),
                    axis=mybir.AxisListType.X)
                return asd

            def pack_store(xw_ps, asd, shard, adt, t):
                r = rows_of(t)
                pk = wpool.tile([P, ROW], bf16, tag="pack")
                nc.vector.tensor_copy(out=pk[:, 0:HF], in_=xw_ps[:])
                if asd is not None:
                    nc.vector.tensor_copy(out=pk[:, HF:HF + 8], in_=asd[:])
                    nc.sync.dma_start(out=adt[t * P:(t + 1) * P, :], in_=asd[:])
                    nc.sync.dma_start(out=shard[t * P:t * P + r, :], in_=pk[:r, :])
                else:
                    nc.sync.dma_start(out=shard[t * P:t * P + r, :], in_=pk[:r, 0:HF])

            # ---------------- P1: xw1 = x @ W1 (+ logits), shard + gather prep
            for t in range(W):
                xw_ps = pspool.tile([P, HF], f32, tag="ps")
                nc.tensor.matmul(xw_ps[:], lhsT=xT_sb[:, t * P:t * P + P],
                                 rhs=W1_sb[:], start=True, stop=True)
                asd = alpha_sd(xw_ps, ac1_r)
                pack_store(xw_ps, asd, xw1_sh, ad1, t)

            nc.gpsimd.collective_compute(
                "AllGather", mybir.AluOpType.bypass, replica_groups=RG,
                ins=[xw1_sh[:]], outs=[xw1_ext[:]])

            # ---------------- MLP branch (overlaps AllGather)
            fragB = fragpool.tile([G, PCA + POS], f32, tag="fragB")
            for t in range(W):
                r = rows_of(t)
                p1 = pspool.tile([P, 128], f32, tag="ps")
                nc.tensor.matmul(p1[:], lhsT=xT_sb[:, t * P:t * P + P],
                                 rhs=pW1_sb[:], start=True, stop=True)
                h1 = wpool.tile([P, 128], f32, tag="mlph1")
                nc.vector.tensor_mul(h1[:], p1[:], g1_r[:])
                nc.vector.tensor_add(h1[:], h1[:], c1_r[:])
                nc.vector.tensor_scalar_max(h1[:], h1[:], 0.0)
                tp1 = pspool.tile([P, 128], f32, tag="ps")
                nc.tensor.transpose(tp1[:], h1[:], ident[:])
                h1T = wpool.tile([P, 128], f32, tag="mlph1T")
                nc.vector.tensor_copy(out=h1T[:], in_=tp1[:])
                p2 = pspool.tile([P, PCA], f32, tag="ps")
                nc.tensor.matmul(p2[:], lhsT=h1T[:], rhs=pW2_sb[:],
                                 start=True, stop=True)
                h2 = wpool.tile([P, PCA], f32, tag="mlph2")
                nc.vector.tensor_mul(h2[:], p2[:], g2_r[:])
                nc.vector.tensor_add(h2[:], h2[:], c2_r[:])
                nc.vector.tensor_scalar_max(h2[:], h2[:], 0.0)
                tp2 = pspool.tile([P, PCA], f32, tag="ps")
                for cch in range(2):
                    nc.tensor.transpose(tp2[:, cch * P:cch * P + P],
                                        h2[:, cch * P:cch * P + P], ident[:])
                h2T = wpool.tile([P, PCA], f32, tag="mlph2T")
                nc.vector.tensor_copy(out=h2T[:], in_=tp2[:])
                p3 = pspool.tile([P, PCA], f32, tag="ps")
                for cch in range(2):
                    nc.tensor.matmul(p3[:], lhsT=h2T[:, cch * P:cch * P + P],
                                     rhs=pW3_sb[:, cch * PCA:(cch + 1) * PCA],
                                     start=(cch == 0), stop=(cch == 1))
                mlp_out = wpool.tile([P, PCA + POS], f32, tag="mlpout")
                nc.vector.tensor_add(mlp_out[:, 0:PCA], p3[:], pb3_r[:])
                # scale encoder
                q1 = pspool.tile([P, 32], f32, tag="ps")
                nc.tensor.matmul(q1[:], lhsT=sT_sb[:, t * P:t * P + P],
                                 rhs=sW1_sb[:], start=True, stop=True)
                r1 = wpool.tile([P, 32], f32, tag="mlpr1")
                nc.vector.tensor_add(r1[:], q1[:], sb1_r[:, 0:32])
                nc.vector.tensor_scalar_max(r1[:], r1[:], 0.0)
                tq = pspool.tile([32, P], f32, tag="ps")
                nc.tensor.transpose(tq[:], r1[:], ident[:])
                r1T = wpool.tile([32, P], f32, tag="mlpr1T")
                nc.vector.tensor_copy(out=r1T[:], in_=tq[:])
                q2 = pspool.tile([P, POS], f32, tag="ps")
                nc.tensor.matmul(q2[:], lhsT=r1T[:], rhs=sW2_sb[:],
                                 start=True, stop=True)
                nc.vector.tensor_add(mlp_out[:, PCA:], q2[:], sb2_r[:])
                nc.tensor.matmul(fragB[:], lhsT=sel_sb[:, t * G:(t + 1) * G],
                                 rhs=mlp_out[:], start=(t == 0), stop=(t == W - 1))
                nc.sync.dma_start(out=hcomb[t * P:t * P + r, HF:],
                                  in_=mlp_out[:r, :])

            # ---------------- GAT aggregation phase
            def gat_aggregate(xw_ext, adt, b_rep, layer):
                for w in range(W):
                    msg = wpool.tile([P, B * ROW], bf16, tag="msg")
                    ad_t = wpool.tile([P, B * 8], f32, tag="adg")
                    for b in range(B):
                        nc.gpsimd.indirect_dma_start(
                            out=msg[:, b * ROW:(b + 1) * ROW], out_offset=None,
                            in_=xw_ext[:],
                            in_offset=bass.IndirectOffsetOnAxis(
                                ap=idxs_sb[:, w * B + b:w * B + b + 1], axis=0))
                        nc.gpsimd.indirect_dma_start(
                            out=ad_t[:, b * 8:(b + 1) * 8], out_offset=None,
                            in_=adt[:],
                            in_offset=bass.IndirectOffsetOnAxis(
                                ap=idxd_sb[:, w * B + b:w * B + b + 1], axis=0))
                    S_sb = wpool.tile([P, B * P], bf16, tag="S")
                    nc.sync.dma_start(
                        out=S_sb[:].rearrange("p (b n) -> p b n", b=B),
                        in_=inp["S_gat"][w * B:(w + 1) * B, :, :]
                            .rearrange("b p n -> p b n"))
                    # e = alpha_src[src] + alpha_dst[dst]; ex = exp(lrelu(e))
                    ee = wpool.tile([P, B * H], f32, tag="ee")
                    nc.vector.tensor_add(
                        ee[:].rearrange("p (b h) -> p b h", h=H),
                        msg[:].rearrange("p (b c) -> p b c", c=ROW)[:, :, HF:HF + H],
                        ad_t[:].rearrange("p (b c) -> p b c", c=8)[:, :, 4:8])
                    nc.vector.scalar_tensor_tensor(
                        out=ee[:], in0=ee[:], scalar=0.2, in1=ee[:],
                        op0=mybir.AluOpType.mult, op1=mybir.AluOpType.max)
                    ex = wpool.tile([P, B * H], bf16, tag="ex")
                    nc.scalar.activation(ex[:], ee[:],
                                         mybir.ActivationFunctionType.Exp)
                    # scale messages by ex (4 blocks per op)
                    sc_tiles = []
                    for gset in range(0, B, 4):
                        nb = min(4, B - gset)
                        sc = scpool.tile([P, 4 * HF], bf16, tag="scm")
                        nc.vector.tensor_mul(
                            sc[:, 0:nb * HF].rearrange("p (b h f) -> p b h f",
                                                       h=H, f=F),
                            msg[:].rearrange("p (b c) -> p b c", c=ROW)
                               [:, gset:gset + nb, 0:HF]
                               .rearrange("p b (h f) -> p b h f", f=F),
                            ex[:, gset * H:(gset + nb) * H]
                               .rearrange("p (b h) -> p b h", h=H)
                               .to_broadcast([P, nb, H, F]))
                        sc_tiles.append(sc)
                    num_ps = pspool.tile([P, HF], f32, tag="ps")
                    den_ps = pspool.tile([P, H], f32, tag="ps")
                    for b in range(B):
                        sc = sc_tiles[b // 4]
                        nc.tensor.matmul(
                            num_ps[:], lhsT=S_sb[:, b * P:(b + 1) * P],
                            rhs=sc[:, (b % 4) * HF:(b % 4 + 1) * HF],
                            start=(b == 0), stop=(b == B - 1))
                        nc.tensor.matmul(
                            den_ps[:], lhsT=S_sb[:, b * P:(b + 1) * P],
                            rhs=ex[:, b * H:(b + 1) * H],
                            start=(b == 0), stop=(b == B - 1))
                    rec = wpool.tile([P, H], f32, tag="rec")
                    nc.vector.tensor_scalar_add(rec[:], den_ps[:], 1e-30)
                    nc.vector.reciprocal(rec[:], rec[:])
                    hw = hslab[:, w * HF:(w + 1) * HF]
                    nc.vector.tensor_mul(
                        hw.rearrange("p (h f) -> p h f", f=F),
                        num_ps[:].rearrange("p (h f) -> p h f", f=F),
                        rec[:].to_broadcast([P, H, F]))
                    nc.vector.tensor_add(hw, hw, b_rep[:])
                    nc.vector.tensor_scalar_max(hw, hw, 0.0)

            gat_aggregate(xw1_ext, ad1, b1_r, 1)

            # ---------------- P2: xw2 = h1 @ W2 (+ logits)
            def prep_layer(W_sb, shard, adt, ac_rep):
                for t in range(W):
                    tp = pspool.tile([P, HF], f32, tag="ps")
                    for cch in range(4):
                        nc.tensor.transpose(
                            tp[:, cch * P:(cch + 1) * P],
                            hslab[:, t * HF + cch * P:t * HF + (cch + 1) * P],
                            ident[:])
                    hT = wpool.tile([P, HF], bf16, tag="hT")
                    nc.vector.tensor_copy(out=hT[:], in_=tp[:])
                    xw_ps = pspool.tile([P, HF], f32, tag="ps")
                    for cch in range(4):
                        nc.tensor.matmul(
                            xw_ps[:], lhsT=hT[:, cch * P:(cch + 1) * P],
                            rhs=W_sb[:, cch * HF:(cch + 1) * HF],
                            start=(cch == 0), stop=(cch == 3))
                    if ac_rep is not None:
                        asd = alpha_sd(xw_ps, ac_rep)
                        pack_store(xw_ps, asd, shard, adt, t)
                    else:
                        pack_store(xw_ps, None, shard, None, t)

            prep_layer(W2_sb, xw2_sh, ad2, ac2_r)
            nc.gpsimd.collective_compute(
                "AllGather", mybir.AluOpType.bypass, replica_groups=RG,
                ins=[xw2_sh[:]], outs=[xw2_ext[:]])

            gat_aggregate(xw2_ext, ad2, b2_r, 2)

            # ---------------- P3: xw3 = h2 @ W3
            prep_layer(W3_sb, xw3_sh, None, None)
            nc.gpsimd.collective_compute(
                "AllGather", mybir.AluOpType.bypass, replica_groups=RG,
                ins=[xw3_sh[:]], outs=[xw3_ext[:]])

            # ---------------- A3: GCN aggregation (norm folded into S_gcn)
            fragA = fragpool.tile([G, HF], f32, tag="fragA")
            for w in range(W):
                r = rows_of(w)
                msg = wpool.tile([P, B * HF], bf16, tag="msg")
                for b in range(B):
                    nc.gpsimd.indirect_dma_start(
                        out=msg[:, b * HF:(b + 1) * HF], out_offset=None,
                        in_=xw3_ext[:],
                        in_offset=bass.IndirectOffsetOnAxis(
                            ap=idxs_sb[:, w * B + b:w * B + b + 1], axis=0))
                S_sb = wpool.tile([P, B * P], bf16, tag="S")
                nc.sync.dma_start(
                    out=S_sb[:].rearrange("p (b n) -> p b n", b=B),
                    in_=inp["S_gcn"][w * B:(w + 1) * B, :, :]
                        .rearrange("b p n -> p b n"))
                num_ps = pspool.tile([P, HF], f32, tag="ps")
                for b in range(B):
                    nc.tensor.matmul(
                        num_ps[:], lhsT=S_sb[:, b * P:(b + 1) * P],
                        rhs=msg[:, b * HF:(b + 1) * HF],
                        start=(b == 0), stop=(b == B - 1))
                ht = wpool.tile([P, HF], f32, tag="htopo")
                nc.vector.tensor_add(ht[:], num_ps[:], b3_r[:])
                nc.tensor.matmul(fragA[:], lhsT=sel_sb[:, w * G:(w + 1) * G],
                                 rhs=ht[:], start=(w == 0), stop=(w == W - 1))
                nc.sync.dma_start(out=hcomb[w * P:w * P + r, 0:HF],
                                  in_=ht[:r, :])

            # ---------------- frag: AllReduce partial segment sums, then mean
            fr = wpool.tile([G, OUTD], f32, tag="frs")
            nc.vector.tensor_copy(out=fr[:, 0:HF], in_=fragA[:])
            nc.vector.tensor_copy(out=fr[:, HF:], in_=fragB[:])
            nc.sync.dma_start(out=frag_in[:], in_=fr[:])
            nc.gpsimd.collective_compute(
                "AllReduce", mybir.AluOpType.add, replica_groups=RG,
                ins=[frag_in[:]], outs=[frag_out[:]])
            fr2 = wpool.tile([G, OUTD], f32, tag="fr2")
            nc.gpsimd.dma_start(out=fr2[:], in_=frag_out[:])
            nc.vector.tensor_scalar_mul(fr2[:], fr2[:], inv_sb[:, 0:1])
            nc.sync.dma_start(out=frag[:], in_=fr2[:])

    nc.compile()
    return nc


WEIGHTS_META = [
    ("W1", (9, HF)), ("W2", (HF, HF)), ("W3", (HF, HF)),
    ("b1", (1, HF)), ("b2", (1, HF)), ("b3", (1, HF)),
    ("a_comb1", (1, 2 * HF)), ("a_comb2", (1, 2 * HF)),
    ("pW1", (9, 128)), ("pb1", (1, 128)), ("g1", (1, 128)), ("be1", (1, 128)),
    ("pW2", (128, 256)), ("pb2", (1, 256)), ("g2", (1, 256)), ("be2", (1, 256)),
    ("pW3", (256, 256)), ("pb3", (1, 256)),
    ("sW1", (1, 32)), ("sb1", (1, 32)), ("sW2", (32, 128)), ("sb2", (1, 128)),
]


def make_in_maps(inputs, per_core, inv_cnt):
    """Build per-core input maps (weights replicated, index data sharded)."""
    f = lambda k: np.ascontiguousarray(np.asarray(inputs[k], np.float32))
    shared = {
        "W1": f("W1"), "W2": f("W2"), "W3": f("W3"),
        "b1": f("b1").reshape(1, HF), "b2": f("b2").reshape(1, HF),
        "b3": f("b3").reshape(1, HF),
        "a_comb1": np.concatenate(
            [f("a_src1").reshape(1, HF), f("a_dst1").reshape(1, HF)], 1),
        "a_comb2": np.concatenate(
            [f("a_src2").reshape(1, HF), f("a_dst2").reshape(1, HF)], 1),
        "pW1": f("pW1"), "pb1": f("pb1").reshape(1, 128),
        "g1": f("g1").reshape(1, 128), "be1": f("be1").reshape(1, 128),
        "pW2": f("pW2"), "pb2": f("pb2").reshape(1, 256),
        "g2": f("g2").reshape(1, 256), "be2": f("be2").reshape(1, 256),
        "pW3": f("pW3"), "pb3": f("pb3").reshape(1, 256),
        "sW1": f("sW1").reshape(1, 32), "sb1": f("sb1").reshape(1, 32),
        "sW2": f("sW2"), "sb2": f("sb2").reshape(1, 128),
        "inv_cnt": inv_cnt,
    }
    in_maps = []
    for pc in per_core:
        m = dict(shared)
        m.update(pc)
        in_maps.append(m)
    return in_maps


def kernel(**inputs):
    x = np.asarray(inputs["x"], np.float32)
    edge_index = np.asarray(inputs["edge_index"], np.int64)
    batch = np.asarray(inputs["batch"], np.int64)
    scales = np.asarray(inputs["scales"], np.float32)
    N = x.shape[0]

    per_core, inv_cnt, npc, W, B = host_prep(x, edge_index, batch, scales)
    nc = build_program(N, npc, W, B, WEIGHTS_META)
    in_maps = make_in_maps(inputs, per_core, inv_cnt)
    res = run_bass_kernel_spmd(nc, in_maps, list(range(NCORES)))
    h_combined = np.concatenate([r["hcomb"] for r in res.results], 0)
    frag = res.results[0]["frag"]
    return frag.astype(np.float32), h_combined.astype(np.float32)


# revision 11
# speedup vs baseline: 1.2190x; 1.2190x over previous
"""Trainium2 Bass kernel for nn_ClusterFeatureBranch (GAT x2 + GCN + MLP branches).

Sharding: nodes (and their in-edges, edge-cut by dst) are partitioned across
8 NeuronCores; small weight matrices are replicated; per-layer transformed
node features are AllGathered so every core can gather messages for its own
edges from local HBM.

Per GAT/GCN layer, per core:
  - prep: xw = h @ W (PE), attention logits a_src/a_dst per node (DVE),
    pack to bf16, DMA to DRAM shard, AllGather -> full [N, 520] bf16.
  - aggregate, per 128-dst-node window: indirect-DMA gather of per-edge
    message rows (by src id), gather of per-edge alpha_dst (by dst id),
    softmax over dst segments via one-hot matmuls (scatter-add on PE into
    PSUM), normalize + bias + relu on DVE.
Softmax max-subtraction is skipped (m=0): logits here are O(0.1), so
exp never overflows and the result is mathematically identical.
"""

import os
import sys

for _p in ("/opt/trn_rl_repo",):
    if os.path.isdir(_p) and _p not in sys.path:
        sys.path.append(_p)

import numpy as np
import ml_dtypes

from concourse import bacc, bass, mybir, tile
from concourse.bass_utils import run_bass_kernel_spmd
from concourse.masks import make_identity

P = 128
H, F = 4, 128
HF = H * F            # 512
ROW = 640             # gather row: 512 feat bf16 + 4 alpha_src + pad (256B mult)
ADW = 64              # alpha row width in f32 (256B)
G = 8
PCA, POS = 256, 128
OUTD = HF + PCA + POS  # 896
BN_EPS = 1e-5
NCORES = 8

f32 = mybir.dt.float32
bf16 = mybir.dt.bfloat16
i32 = mybir.dt.int32
bf16np = ml_dtypes.bfloat16


# ---------------------------------------------------------------- host prep

def _padT(a, cols):
    out = np.zeros((a.shape[1], cols), np.float32)
    out[:, :a.shape[0]] = a.T
    return out


def host_prep(x, edge_index, batch, scales, n_cores=NCORES):
    """Pure index/layout preprocessing (the sharding step). Returns per-core
    tensors + shared scalars. All float math stays on device."""
    N = x.shape[0]
    npc = N // n_cores
    W = (npc + P - 1) // P

    src = np.asarray(edge_index[0], np.int64)
    dst = np.asarray(edge_index[1], np.int64)
    loops = np.arange(N, dtype=np.int64)
    src = np.concatenate([src, loops])
    dst = np.concatenate([dst, loops])

    # gcn norm (pure graph structure): deg includes self loop
    deg = np.bincount(dst, minlength=N).astype(np.float64)
    dis = np.where(deg > 0, 1.0 / np.sqrt(deg), 0.0)
    enorm = (dis[src] * dis[dst]).astype(np.float32)

    order = np.argsort(dst, kind="stable")
    src_s, dst_s, enorm_s = src[order], dst[order], enorm[order]

    # split per core / per window, find uniform B
    counts = np.zeros((n_cores, W), np.int64)
    core_of = dst_s // npc
    win_of = (dst_s - core_of * npc) // P
    for c in range(n_cores):
        m = core_of == c
        cw = win_of[m]
        for w in range(W):
            counts[c, w] = int((cw == w).sum())
    B = int(max(1, int(np.ceil(counts.max() / P))))
    EPW = B * P

    per_core = []
    gcounts = np.bincount(np.asarray(batch, np.int64), minlength=G).astype(np.float32)
    inv_cnt = (1.0 / np.maximum(gcounts, 1.0)).astype(np.float32).reshape(G, 1)

    for c in range(n_cores):
        m = core_of == c
        cs, cd, cn = src_s[m], dst_s[m], enorm_s[m]
        cw = win_of[m]
        X = B * 8  # idx cols per window in 16-wrap layout
        g_src = np.zeros((16, W * X), np.int16)
        g_dst = np.zeros((16, W * X), np.int16)
        S_gat = np.zeros((W * B, P, P), np.float32)
        S_gcn = np.zeros((W * B, P, P), np.float32)
        for w in range(W):
            wm = cw == w
            es, ed, en = cs[wm], cd[wm], cn[wm]
            k = len(es)
            assert k <= EPW
            e = np.arange(k)
            b, p = e // P, e % P
            blk = w * B + b
            g_src[e % 16, w * X + e // 16] = es
            g_dst[e % 16, w * X + e // 16] = (ed - c * npc).astype(np.int16)
            nloc = (ed - c * npc - w * P).astype(np.int64)
            S_gat[blk, p, nloc] = 1.0
            S_gcn[blk, p, nloc] = en
        # batch one-hot selector [P, W*G] and xT/sT shards
        sel = np.zeros((P, W * G), np.float32)
        nb = np.asarray(batch, np.int64)[c * npc:(c + 1) * npc]
        for w in range(W):
            rows = min(P, npc - w * P)
            sel[np.arange(rows), w * G + nb[w * P:w * P + rows]] = 1.0
        per_core.append(dict(
            idx_src=np.tile(g_src, (8, 1)),
            idx_dst=np.tile(g_dst, (8, 1)),
            S_gat=S_gat.astype(bf16np),
            S_gcn=S_gcn.astype(bf16np),
            sel=sel,
            xT=_padT(np.asarray(x, np.float32)[c * npc:(c + 1) * npc], W * P),
            sT=_padT(np.asarray(scales, np.float32)[c * npc:(c + 1) * npc], W * P),
        ))
    return per_core, inv_cnt, npc, W, B


# ---------------------------------------------------------------- program

def build_program(N, npc, W, B, weights_meta):
    """Build the single-program-multiple-data Bass module."""
    nc = bacc.Bacc()
    NPAD = W * P

    # ---- I/O
    inp = {}
    def din(name, shape, dt):
        inp[name] = nc.declare_dram_parameter(name, list(shape), dt, isOutput=False)
        return inp[name]

    din("idx_src", (P, W * B * 8), mybir.dt.int16)
    din("idx_dst", (P, W * B * 8), mybir.dt.int16)
    din("S_gat", (W * B, P, P), bf16)
    din("S_gcn", (W * B, P, P), bf16)
    din("sel", (P, W * G), f32)
    din("xT", (9, W * P), f32)
    din("sT", (1, W * P), f32)
    din("inv_cnt", (G, 1), f32)
    for name, shape in weights_meta:
        din(name, shape, f32)

    hcomb = nc.declare_dram_parameter("hcomb", [npc, OUTD], f32, isOutput=True)
    frag = nc.declare_dram_parameter("frag", [G, OUTD], f32, isOutput=True)

    # ---- internal DRAM
    xw1_sh = nc.dram_tensor("xw1_sh", [npc, ROW], bf16)
    xw2_sh = nc.dram_tensor("xw2_sh", [npc, ROW], bf16)
    xw3_sh = nc.dram_tensor("xw3_sh", [npc, HF], bf16)
    xw1_ext = nc.dram_tensor("xw1_ext", [N, ROW], bf16, addr_space="Shared")
    xw2_ext = nc.dram_tensor("xw2_ext", [N, ROW], bf16, addr_space="Shared")
    xw3_ext = nc.dram_tensor("xw3_ext", [N, HF], bf16, addr_space="Shared")
    ad1 = nc.dram_tensor("ad1", [NPAD, ADW], f32)
    ad2 = nc.dram_tensor("ad2", [NPAD, ADW], f32)
    frag_in = nc.dram_tensor("frag_in", [G, OUTD], f32)
    frag_out = nc.dram_tensor("frag_out", [G, OUTD], f32, addr_space="Shared")

    RG = [list(range(NCORES))]

    with tile.TileContext(nc) as tc:
        with (
            tc.tile_pool(name="const", bufs=1) as cpool,
            tc.tile_pool(name="resid", bufs=1) as rpool,
            tc.tile_pool(name="work", bufs=2) as wpool,
            tc.tile_pool(name="sc", bufs=2) as scpool,
            tc.tile_pool(name="ps", bufs=4, space="PSUM") as pspool,
            tc.tile_pool(name="psfrag", bufs=1, space="PSUM") as fragpool,
        ):
            # ---------------- constants
            ident = cpool.tile([P, P], f32)
            make_identity(nc, ident[:])
            ones = cpool.tile([1, P], f32)
            nc.vector.memset(ones[:], 1.0)
            ident_bf = cpool.tile([P, P], bf16)
            nc.vector.tensor_copy(out=ident_bf[:], in_=ident[:])

            def rep_row(name, d, cast=None, scale=None):
                """replicate a [1, d] dram row across 128 partitions"""
                row = rpool.tile([1, d], f32, tag="reprow")
                nc.sync.dma_start(out=row[:], in_=inp[name][:])
                out = cpool.tile([P, d], f32, tag=f"rep_{name}")
                for j in range(0, d, 512):
                    e = min(512, d - j)
                    pr = pspool.tile([P, 512], f32, tag="ps")
                    nc.tensor.matmul(pr[:, :e], lhsT=ones[:], rhs=row[:, j:j + e],
                                     start=True, stop=True)
                    nc.vector.tensor_copy(out=out[:, j:j + e], in_=pr[:, :e])
                if scale is not None:
                    nc.vector.tensor_scalar_mul(out[:], out[:], scale)
                return out

            bns = float(1.0 / np.sqrt(1.0 + BN_EPS))
            b1_r = rep_row("b1", HF)
            b2_r = rep_row("b2", HF)
            b3_r = rep_row("b3", HF)
            ac1_r = rep_row("a_comb1", 2 * HF)
            ac2_r = rep_row("a_comb2", 2 * HF)
            g1_r = rep_row("g1", 128, scale=bns)
            be1_r = rep_row("be1", 128)
            pb1_r = rep_row("pb1", 128)
            g2_r = rep_row("g2", 256, scale=bns)
            be2_r = rep_row("be2", 256)
            pb2_r = rep_row("pb2", 256)
            pb3_r = rep_row("pb3", 256)
            sb1_r = rep_row("sb1", 32)
            sb2_r = rep_row("sb2", 128)
            # BN folded consts: c = pb*s + be
            c1_r = cpool.tile([P, 128], f32)
            nc.vector.tensor_mul(c1_r[:], pb1_r[:], g1_r[:])
            nc.vector.tensor_add(c1_r[:], c1_r[:], be1_r[:])
            c2_r = cpool.tile([P, 256], f32)
            nc.vector.tensor_mul(c2_r[:], pb2_r[:], g2_r[:])
            nc.vector.tensor_add(c2_r[:], c2_r[:], be2_r[:])

            # ---------------- resident loads
            hslab = rpool.tile([P, W * HF], bf16, tag="hslab")
            idxs_sb = rpool.tile([P, W * B * 8], mybir.dt.int16, tag="idxs")
            nc.gpsimd.dma_start(out=idxs_sb[:], in_=inp["idx_src"][:])
            idxd_sb = rpool.tile([P, W * B * 8], mybir.dt.int16, tag="idxd")
            nc.gpsimd.dma_start(out=idxd_sb[:], in_=inp["idx_dst"][:])
            sel_sb = rpool.tile([P, W * G], f32, tag="sel")
            nc.sync.dma_start(out=sel_sb[:], in_=inp["sel"][:])
            inv_sb = rpool.tile([G, 1], f32, tag="invc")
            nc.sync.dma_start(out=inv_sb[:], in_=inp["inv_cnt"][:])

            def load_w_bf16(name, kchunks, ncols):
                out = rpool.tile([P, kchunks * ncols], bf16, tag=f"w_{name}")
                for hh in range(2):
                    tmp = wpool.tile([P, kchunks * ncols // 2], f32, tag="asd_tmp")
                    nc.sync.dma_start(
                        out=tmp[:],
                        in_=inp[name][:].rearrange("(c p) n -> p c n", p=P)
                            [:, hh * kchunks // 2:(hh + 1) * kchunks // 2, :])
                    nc.vector.tensor_copy(
                        out=out[:, hh * kchunks * ncols // 2:
                                (hh + 1) * kchunks * ncols // 2], in_=tmp[:])
                return out

            W2_sb = load_w_bf16("W2", 4, HF)
            W3_sb = load_w_bf16("W3", 4, HF)
            W1_sb = rpool.tile([9, HF], f32, tag="W1")
            nc.sync.dma_start(out=W1_sb[:], in_=inp["W1"][:])
            pW2_sb = rpool.tile([P, PCA], f32, tag="pW2")
            nc.sync.dma_start(out=pW2_sb[:], in_=inp["pW2"][:])
            pW3_sb = rpool.tile([P, 2 * PCA], f32, tag="pW3")
            nc.sync.dma_start(
                out=pW3_sb[:].rearrange("p (c n) -> p c n", c=2),
                in_=inp["pW3"][:].rearrange("(c p) n -> p c n", p=P))
            pW1_sb = rpool.tile([9, 128], f32, tag="pW1")
            nc.sync.dma_start(out=pW1_sb[:], in_=inp["pW1"][:])
            sW1_sb = rpool.tile([1, 32], f32, tag="sW1")
            nc.sync.dma_start(out=sW1_sb[:], in_=inp["sW1"][:])
            sW2_sb = rpool.tile([32, 128], f32, tag="sW2")
            nc.sync.dma_start(out=sW2_sb[:], in_=inp["sW2"][:])
            xT_sb = rpool.tile([9, W * P], f32, tag="xT")
            nc.sync.dma_start(out=xT_sb[:], in_=inp["xT"][:])
            sT_sb = rpool.tile([1, W * P], f32, tag="sT")
            nc.sync.dma_start(out=sT_sb[:], in_=inp["sT"][:])

            def rows_of(t):
                return min(P, npc - t * P)

            # -------- attention logit helper: asd [P, 8] from xw psum
            def alpha_sd(xw_ps, ac_rep):
                tmp = wpool.tile([P, 2 * HF], f32, tag="asd_tmp")
                nc.vector.tensor_mul(tmp[:, 0:HF], xw_ps[:], ac_rep[:, 0:HF])
                nc.vector.tensor_mul(tmp[:, HF:], xw_ps[:], ac_rep[:, HF:])
                asd = wpool.tile([P, 8], f32, tag="asd")
                nc.vector.reduce_sum(
                    out=asd[:],
                    in_=tmp[:].rearrange("p (c f) -> p c f", f=F),
                    axis=mybir.AxisListType.X)
                return asd

            def pack_store(xw_ps, asd, shard, adt, t):
                r = rows_of(t)
                pk = wpool.tile([P, ROW], bf16, tag="pack")
                nc.vector.tensor_copy(out=pk[:, 0:HF], in_=xw_ps[:])
                if asd is not None:
                    nc.vector.memset(pk[:, HF + 8:], 0.0)
                    nc.vector.tensor_copy(out=pk[:, HF:HF + 8], in_=asd[:])
                    a64 = wpool.tile([P, ADW], f32, tag="a64")
                    nc.vector.memset(a64[:, 8:], 0.0)
                    nc.vector.tensor_copy(out=a64[:, 0:8], in_=asd[:])
                    nc.sync.dma_start(out=adt[t * P:(t + 1) * P, :], in_=a64[:])
                    nc.sync.dma_start(out=shard[t * P:t * P + r, :], in_=pk[:r, :])
                else:
                    nc.sync.dma_start(out=shard[t * P:t * P + r, :], in_=pk[:r, 0:HF])

            # ---------------- P1: xw1 = x @ W1 (+ logits), shard + gather prep
            for t in range(W):
                xw_ps = pspool.tile([P, HF], f32, tag="ps")
                nc.tensor.matmul(xw_ps[:], lhsT=xT_sb[:, t * P:t * P + P],
                                 rhs=W1_sb[:], start=True, stop=True)
                asd = alpha_sd(xw_ps, ac1_r)
                pack_store(xw_ps, asd, xw1_sh, ad1, t)

            nc.gpsimd.collective_compute(
                "AllGather", mybir.AluOpType.bypass, replica_groups=RG,
                ins=[xw1_sh[:]], outs=[xw1_ext[:]])

            # ---------------- MLP branch (overlaps AllGather)
            fragB = fragpool.tile([G, PCA + POS], f32, tag="fragB")
            for t in range(W):
                r = rows_of(t)
                p1 = pspool.tile([P, 128], f32, tag="ps")
                nc.tensor.matmul(p1[:], lhsT=xT_sb[:, t * P:t * P + P],
                                 rhs=pW1_sb[:], start=True, stop=True)
                h1 = wpool.tile([P, 128], f32, tag="mlph1")
                nc.vector.tensor_mul(h1[:], p1[:], g1_r[:])
                nc.vector.tensor_add(h1[:], h1[:], c1_r[:])
                nc.vector.tensor_scalar_max(h1[:], h1[:], 0.0)
                tp1 = pspool.tile([P, 128], f32, tag="ps")
                nc.tensor.transpose(tp1[:], h1[:], ident[:])
                h1T = wpool.tile([P, 128], f32, tag="mlph1T")
                nc.vector.tensor_copy(out=h1T[:], in_=tp1[:])
                p2 = pspool.tile([P, PCA], f32, tag="ps")
                nc.tensor.matmul(p2[:], lhsT=h1T[:], rhs=pW2_sb[:],
                                 start=True, stop=True)
                h2 = wpool.tile([P, PCA], f32, tag="mlph2")
                nc.vector.tensor_mul(h2[:], p2[:], g2_r[:])
                nc.vector.tensor_add(h2[:], h2[:], c2_r[:])
                nc.vector.tensor_scalar_max(h2[:], h2[:], 0.0)
                tp2 = pspool.tile([P, PCA], f32, tag="ps")
                for cch in range(2):
                    nc.tensor.transpose(tp2[:, cch * P:cch * P + P],
                                        h2[:, cch * P:cch * P + P], ident[:])
                h2T = wpool.tile([P, PCA], f32, tag="mlph2T")
                nc.vector.tensor_copy(out=h2T[:], in_=tp2[:])
                p3 = pspool.tile([P, PCA], f32, tag="ps")
                for cch in range(2):
                    nc.tensor.matmul(p3[:], lhsT=h2T[:, cch * P:cch * P + P],
                                     rhs=pW3_sb[:, cch * PCA:(cch + 1) * PCA],
                                     start=(cch == 0), stop=(cch == 1))
                mlp_out = wpool.tile([P, PCA + POS], f32, tag="mlpout")
                nc.vector.tensor_add(mlp_out[:, 0:PCA], p3[:], pb3_r[:])
                # scale encoder
                q1 = pspool.tile([P, 32], f32, tag="ps")
                nc.tensor.matmul(q1[:], lhsT=sT_sb[:, t * P:t * P + P],
                                 rhs=sW1_sb[:], start=True, stop=True)
                r1 = wpool.tile([P, 32], f32, tag="mlpr1")
                nc.vector.tensor_add(r1[:], q1[:], sb1_r[:, 0:32])
                nc.vector.tensor_scalar_max(r1[:], r1[:], 0.0)
                tq = pspool.tile([32, P], f32, tag="ps")
                nc.tensor.transpose(tq[:], r1[:], ident[:])
                r1T = wpool.tile([32, P], f32, tag="mlpr1T")
                nc.vector.tensor_copy(out=r1T[:], in_=tq[:])
                q2 = pspool.tile([P, POS], f32, tag="ps")
                nc.tensor.matmul(q2[:], lhsT=r1T[:], rhs=sW2_sb[:],
                                 start=True, stop=True)
                nc.vector.tensor_add(mlp_out[:, PCA:], q2[:], sb2_r[:])
                nc.tensor.matmul(fragB[:], lhsT=sel_sb[:, t * G:(t + 1) * G],
                                 rhs=mlp_out[:], start=(t == 0), stop=(t == W - 1))
                nc.sync.dma_start(out=hcomb[t * P:t * P + r, HF:],
                                  in_=mlp_out[:r, :])

            # ---------------- GAT aggregation phase
            def gat_aggregate(xw_ext, adt, b_rep, layer):
                X = B * 8
                EPW = B * P
                for w in range(W):
                    msg = wpool.tile([P, B * ROW], bf16, tag="msg")
                    ad_t = wpool.tile([P, B * ADW], f32, tag="adg")
                    nc.gpsimd.dma_gather(
                        out_ap=msg[:].rearrange("p (b c) -> p b c", c=ROW),
                        in_ap=xw_ext[:], idxs_ap=idxs_sb[:, w * X:(w + 1) * X],
                        num_idxs=EPW, num_idxs_reg=EPW, elem_size=ROW,
                        single_packet=False)
                    nc.gpsimd.dma_gather(
                        out_ap=ad_t[:].rearrange("p (b c) -> p b c", c=ADW),
                        in_ap=adt[:], idxs_ap=idxd_sb[:, w * X:(w + 1) * X],
                        num_idxs=EPW, num_idxs_reg=EPW, elem_size=ADW,
                        single_packet=False)
                    S_sb = wpool.tile([P, B * P], bf16, tag="S")
                    nc.sync.dma_start(
                        out=S_sb[:].rearrange("p (b n) -> p b n", b=B),
                        in_=inp["S_gat"][w * B:(w + 1) * B, :, :]
                            .rearrange("b p n -> p b n"))
                    # e = alpha_src[src] + alpha_dst[dst]; ex = exp(lrelu(e))
                    ee = wpool.tile([P, B * H], f32, tag="ee")
                    nc.vector.tensor_add(
                        ee[:].rearrange("p (b h) -> p b h", h=H),
                        msg[:].rearrange("p (b c) -> p b c", c=ROW)[:, :, HF:HF + H],
                        ad_t[:].rearrange("p (b c) -> p b c", c=ADW)[:, :, 4:8])
                    nc.vector.scalar_tensor_tensor(
                        out=ee[:], in0=ee[:], scalar=0.2, in1=ee[:],
                        op0=mybir.AluOpType.mult, op1=mybir.AluOpType.max)
                    ex = wpool.tile([P, B * H], bf16, tag="ex")
                    nc.scalar.activation(ex[:], ee[:],
                                         mybir.ActivationFunctionType.Exp)
                    # scale messages by ex (4 blocks per op)
                    sc_tiles = []
                    for gset in range(0, B, 4):
                        nb = min(4, B - gset)
                        sc = scpool.tile([P, 4 * HF], bf16, tag="scm")
                        nc.vector.tensor_mul(
                            sc[:, 0:nb * HF].rearrange("p (b h f) -> p b h f",
                                                       h=H, f=F),
                            msg[:].rearrange("p (b c) -> p b c", c=ROW)
                               [:, gset:gset + nb, 0:HF]
                               .rearrange("p b (h f) -> p b h f", f=F),
                            ex[:, gset * H:(gset + nb) * H]
                               .rearrange("p (b h) -> p b h", h=H)
                               .to_broadcast([P, nb, H, F]))
                        sc_tiles.append(sc)
                    num_ps = pspool.tile([P, HF], f32, tag="ps")
                    den_ps = pspool.tile([P, H], f32, tag="ps")
                    for b in range(B):
                        sc = sc_tiles[b // 4]
                        nc.tensor.matmul(
                            num_ps[:], lhsT=S_sb[:, b * P:(b + 1) * P],
                            rhs=sc[:, (b % 4) * HF:(b % 4 + 1) * HF],
                            start=(b == 0), stop=(b == B - 1))
                        nc.tensor.matmul(
                            den_ps[:], lhsT=S_sb[:, b * P:(b + 1) * P],
                            rhs=ex[:, b * H:(b + 1) * H],
                            start=(b == 0), stop=(b == B - 1))
                    rec = wpool.tile([P, H], f32, tag="rec")
                    nc.vector.tensor_scalar_add(rec[:], den_ps[:], 1e-30)
                    nc.vector.reciprocal(rec[:], rec[:])
                    hw = hslab[:, w * HF:(w + 1) * HF]
                    nc.vector.tensor_mul(
                        hw.rearrange("p (h f) -> p h f", f=F),
                        num_ps[:].rearrange("p (h f) -> p h f", f=F),
                        rec[:].to_broadcast([P, H, F]))
                    nc.vector.tensor_add(hw, hw, b_rep[:])
                    nc.vector.tensor_scalar_max(hw, hw, 0.0)

            gat_aggregate(xw1_ext, ad1, b1_r, 1)

            # ---------------- P2: xw2 = h1 @ W2 (+ logits)
            def prep_layer(W_sb, shard, adt, ac_rep):
                for t in range(W):
                    tp = pspool.tile([P, HF], bf16, tag="ps")
                    for cch in range(4):
                        nc.tensor.transpose(
                            tp[:, cch * P:(cch + 1) * P],
                            hslab[:, t * HF + cch * P:t * HF + (cch + 1) * P],
                            ident_bf[:])
                    hT = wpool.tile([P, HF], bf16, tag="hT")
                    nc.vector.tensor_copy(out=hT[:], in_=tp[:])
                    xw_ps = pspool.tile([P, HF], f32, tag="ps")
                    for cch in range(4):
                        nc.tensor.matmul(
                            xw_ps[:], lhsT=hT[:, cch * P:(cch + 1) * P],
                            rhs=W_sb[:, cch * HF:(cch + 1) * HF],
                            start=(cch == 0), stop=(cch == 3))
                    if ac_rep is not None:
                        asd = alpha_sd(xw_ps, ac_rep)
                        pack_store(xw_ps, asd, shard, adt, t)
                    else:
                        pack_store(xw_ps, None, shard, None, t)

            prep_layer(W2_sb, xw2_sh, ad2, ac2_r)
            nc.gpsimd.collective_compute(
                "AllGather", mybir.AluOpType.bypass, replica_groups=RG,
                ins=[xw2_sh[:]], outs=[xw2_ext[:]])

            gat_aggregate(xw2_ext, ad2, b2_r, 2)

            # ---------------- P3: xw3 = h2 @ W3
            prep_layer(W3_sb, xw3_sh, None, None)
            nc.gpsimd.collective_compute(
                "AllGather", mybir.AluOpType.bypass, replica_groups=RG,
                ins=[xw3_sh[:]], outs=[xw3_ext[:]])

            # ---------------- A3: GCN aggregation (norm folded into S_gcn)
            fragA = fragpool.tile([G, HF], f32, tag="fragA")
            for w in range(W):
                r = rows_of(w)
                msg = wpool.tile([P, B * HF], bf16, tag="msg")
                nc.gpsimd.dma_gather(
                    out_ap=msg[:].rearrange("p (b c) -> p b c", c=HF),
                    in_ap=xw3_ext[:], idxs_ap=idxs_sb[:, w * B * 8:(w + 1) * B * 8],
                    num_idxs=B * P, num_idxs_reg=B * P, elem_size=HF,
                    single_packet=False)
                S_sb = wpool.tile([P, B * P], bf16, tag="S")
                nc.sync.dma_start(
                    out=S_sb[:].rearrange("p (b n) -> p b n", b=B),
                    in_=inp["S_gcn"][w * B:(w + 1) * B, :, :]
                        .rearrange("b p n -> p b n"))
                num_ps = pspool.tile([P, HF], f32, tag="ps")
                for b in range(B):
                    nc.tensor.matmul(
                        num_ps[:], lhsT=S_sb[:, b * P:(b + 1) * P],
                        rhs=msg[:, b * HF:(b + 1) * HF],
                        start=(b == 0), stop=(b == B - 1))
                ht = wpool.tile([P, HF], f32, tag="htopo")
                nc.vector.tensor_add(ht[:], num_ps[:], b3_r[:])
                nc.tensor.matmul(fragA[:], lhsT=sel_sb[:, w * G:(w + 1) * G],
                                 rhs=ht[:], start=(w == 0), stop=(w == W - 1))
                nc.sync.dma_start(out=hcomb[w * P:w * P + r, 0:HF],
                                  in_=ht[:r, :])

            # ---------------- frag: AllReduce partial segment sums, then mean
            fr = wpool.tile([G, OUTD], f32, tag="frs")
            nc.vector.tensor_copy(out=fr[:, 0:HF], in_=fragA[:])
            nc.vector.tensor_copy(out=fr[:, HF:], in_=fragB[:])
            nc.sync.dma_start(out=frag_in[:], in_=fr[:])
            nc.gpsimd.collective_compute(
                "AllReduce", mybir.AluOpType.add, replica_groups=RG,
                ins=[frag_in[:]], outs=[frag_out[:]])
            fr2 = wpool.tile([G, OUTD], f32, tag="fr2")
            nc.gpsimd.dma_start(out=fr2[:], in_=frag_out[:])
            nc.vector.tensor_scalar_mul(fr2[:], fr2[:], inv_sb[:, 0:1])
            nc.sync.dma_start(out=frag[:], in_=fr2[:])

    nc.compile()
    return nc


WEIGHTS_META = [
    ("W1", (9, HF)), ("W2", (HF, HF)), ("W3", (HF, HF)),
    ("b1", (1, HF)), ("b2", (1, HF)), ("b3", (1, HF)),
    ("a_comb1", (1, 2 * HF)), ("a_comb2", (1, 2 * HF)),
    ("pW1", (9, 128)), ("pb1", (1, 128)), ("g1", (1, 128)), ("be1", (1, 128)),
    ("pW2", (128, 256)), ("pb2", (1, 256)), ("g2", (1, 256)), ("be2", (1, 256)),
    ("pW3", (256, 256)), ("pb3", (1, 256)),
    ("sW1", (1, 32)), ("sb1", (1, 32)), ("sW2", (32, 128)), ("sb2", (1, 128)),
]


def make_in_maps(inputs, per_core, inv_cnt):
    """Build per-core input maps (weights replicated, index data sharded)."""
    f = lambda k: np.ascontiguousarray(np.asarray(inputs[k], np.float32))
    shared = {
        "W1": f("W1"), "W2": f("W2"), "W3": f("W3"),
        "b1": f("b1").reshape(1, HF), "b2": f("b2").reshape(1, HF),
        "b3": f("b3").reshape(1, HF),
        "a_comb1": np.concatenate(
            [f("a_src1").reshape(1, HF), f("a_dst1").reshape(1, HF)], 1),
        "a_comb2": np.concatenate(
            [f("a_src2").reshape(1, HF), f("a_dst2").reshape(1, HF)], 1),
        "pW1": f("pW1"), "pb1": f("pb1").reshape(1, 128),
        "g1": f("g1").reshape(1, 128), "be1": f("be1").reshape(1, 128),
        "pW2": f("pW2"), "pb2": f("pb2").reshape(1, 256),
        "g2": f("g2").reshape(1, 256), "be2": f("be2").reshape(1, 256),
        "pW3": f("pW3"), "pb3": f("pb3").reshape(1, 256),
        "sW1": f("sW1").reshape(1, 32), "sb1": f("sb1").reshape(1, 32),
        "sW2": f("sW2"), "sb2": f("sb2").reshape(1, 128),
        "inv_cnt": inv_cnt,
    }
    in_maps = []
    for pc in per_core:
        m = dict(shared)
        m.update(pc)
        in_maps.append(m)
    return in_maps


def kernel(**inputs):
    x = np.asarray(inputs["x"], np.float32)
    edge_index = np.asarray(inputs["edge_index"], np.int64)
    batch = np.asarray(inputs["batch"], np.int64)
    scales = np.asarray(inputs["scales"], np.float32)
    N = x.shape[0]

    per_core, inv_cnt, npc, W, B = host_prep(x, edge_index, batch, scales)
    nc = build_program(N, npc, W, B, WEIGHTS_META)
    in_maps = make_in_maps(inputs, per_core, inv_cnt)
    res = run_bass_kernel_spmd(nc, in_maps, list(range(NCORES)))
    h_combined = np.concatenate([r["hcomb"] for r in res.results], 0)
    frag = res.results[0]["frag"]
    return frag.astype(np.float32), h_combined.astype(np.float32)
